# revision 4
# baseline (speedup 1.0000x reference)
"""Multi-head causal attention with RoPE on 8 Trainium2 NeuronCores (Bass/Tile).

Problem: B=4, T=2048, C=1024, 16 heads x 64. y = softmax(rope(q) rope(k)^T / 8,
causal) v @ Wp + bp with q/k/v = x @ Wq/Wk/Wv.

Sharding: core c handles batch b=c//2, head-group hg=c%2 (8 heads). Each core
computes its QKV projections with column-sliced weights, flash-style causal
attention for its heads, and a partial output projection y^T = Wp_slice^T @
out^T. The host sums the two partials per batch (tensor-parallel reduce) and
adds the bias.

Device layouts (all transposed so no on-device transposes are needed):
  xT (C,T), q^T/k^T (512,T), v (T,520 = 8 heads x [64 dims | 1.0]), y^T (C,T).
  Wq/Wk columns are permuted per head to [even dims | odd dims] so RoPE becomes
  elementwise ops between 32-partition slices (q/k consistent => scores
  unchanged). Scores are computed as S^T (keys on partitions) with 2 heads
  row-packed per matmul (K=64 each); exp runs on ACT out of PSUM with the 1/8
  scale folded in; causal masking is a post-exp affine_select on the 4 diagonal
  blocks; the ones column appended to v makes the AV matmul emit the softmax
  denominator as row 64 of the accumulator; normalization broadcasts 1/denom
  over partitions with a K=1 matmul against a ones row.
"""
from contextlib import ExitStack

import numpy as np

import concourse.bacc as bacc
import concourse.bass as bass
import concourse.mybir as mybir
import concourse.tile as tile
from concourse.bass_utils import run_bass_kernel_spmd

N_HEAD = 16
HS = 64
C = 1024
B = 4
T = 2048
THETA = 10000.0
N_CORES = 8
HL = N_HEAD // 2          # heads per core
DL = HL * HS              # 512 local head dims
QTW = 512                 # query tile width
KBW = 128                 # key block width
F32 = mybir.dt.float32
EXPF = mybir.ActivationFunctionType.Exp


def build_nc(t_len=T):
    assert t_len % QTW == 0
    nqt = t_len // QTW
    nkb_per_qt = QTW // KBW  # 4

    nc = bacc.Bacc("TRN2", target_bir_lowering=False, debug=False,
                   num_devices=N_CORES)
    xT_d = nc.dram_tensor("xT", [C, t_len], F32, kind="ExternalInput")
    wq_d = nc.dram_tensor("wq", [C, DL], F32, kind="ExternalInput")
    wk_d = nc.dram_tensor("wk", [C, DL], F32, kind="ExternalInput")
    wv_d = nc.dram_tensor("wv", [C, DL], F32, kind="ExternalInput")
    wp_d = nc.dram_tensor("wp", [DL, C], F32, kind="ExternalInput")
    cos_d = nc.dram_tensor("cosT", [HS // 2, t_len], F32, kind="ExternalInput")
    sin_d = nc.dram_tensor("sinT", [HS // 2, t_len], F32, kind="ExternalInput")
    yT_d = nc.dram_tensor("yT", [C, t_len], F32, kind="ExternalOutput")

    xT_r = xT_d.ap().rearrange("(cb p) t -> p cb t", p=128)    # (128, 8, t)
    wq_r = wq_d.ap().rearrange("(cb p) m -> p cb m", p=128)    # (128, 8, 512)
    wk_r = wk_d.ap().rearrange("(cb p) m -> p cb m", p=128)
    wv_r = wv_d.ap().rearrange("(cb p) m -> p cb m", p=128)
    wp_r = wp_d.ap().rearrange("(pb p) m -> p pb m", p=128)    # (128, 4, 1024)
    yT_r = yT_d.ap().rearrange("(mb p) t -> p mb t", p=128)    # (128, 8, t)

    with tile.TileContext(nc) as tc, ExitStack() as ctx:
        persist = ctx.enter_context(tc.tile_pool(name="persist", bufs=1))
        xpool = ctx.enter_context(tc.tile_pool(name="xpool", bufs=8))
        cpool = ctx.enter_context(tc.tile_pool(name="cpool", bufs=2))
        qpool = ctx.enter_context(tc.tile_pool(name="qpool", bufs=5))
        opool = ctx.enter_context(tc.tile_pool(name="opool", bufs=5))
        ppool = ctx.enter_context(tc.tile_pool(name="ppool", bufs=4))
        bpool = ctx.enter_context(tc.tile_pool(name="bpool", bufs=2))
        rpool = ctx.enter_context(tc.tile_pool(name="rpool", bufs=2))
        tpool = ctx.enter_context(tc.tile_pool(name="tpool", bufs=4))
        ypool = ctx.enter_context(tc.tile_pool(name="ypool", bufs=3))
        psmm = ctx.enter_context(tc.tile_pool(name="psmm", bufs=4, space="PSUM"))
        psav = ctx.enter_context(tc.tile_pool(name="psav", bufs=3, space="PSUM"))

        wq_sb = persist.tile([128, 8, DL], F32)
        nc.sync.dma_start(wq_sb, wq_r)
        wk_sb = persist.tile([128, 8, DL], F32)
        nc.sync.dma_start(wk_sb, wk_r)
        wv_sb = persist.tile([128, 8, DL], F32)
        nc.sync.dma_start(wv_sb, wv_r)
        wp_sb = persist.tile([128, 4, C], F32)
        nc.sync.dma_start(wp_sb, wp_r)

        kT_sb = persist.tile([128, 4, t_len], F32)           # rope'd k^T
        v_sb = persist.tile([128, t_len // KBW, HL, HS + 1], F32)
        nc.vector.memset(v_sb[:, :, :, HS], 1.0)             # denominator ones
        ones_sb = persist.tile([1, HS], F32)
        nc.vector.memset(ones_sb, 1.0)

        def rope_write(dst_even, dst_odd, ps, g, cos_t, sin_t):
            """dst_even/odd: (32, QTW) SBUF APs; ps: (128, QTW) PSUM tile;
            g: head half (0/1) selecting partitions [64g, 64g+64)."""
            e0, o0 = 64 * g, 64 * g + 32
            t_ec = tpool.tile([32, QTW], F32, tag="ropet", name=f"t_ec_{nc.next_id()}")
            t_os = tpool.tile([32, QTW], F32, tag="ropet", name=f"t_os_{nc.next_id()}")
            nc.vector.tensor_mul(t_ec, ps[e0:e0 + 32, :], cos_t)
            nc.vector.tensor_mul(t_os, ps[o0:o0 + 32, :], sin_t)
            nc.vector.tensor_sub(dst_even, t_ec, t_os)
            t_es = tpool.tile([32, QTW], F32, tag="ropet", name=f"t_es_{nc.next_id()}")
            t_oc = tpool.tile([32, QTW], F32, tag="ropet", name=f"t_oc_{nc.next_id()}")
            nc.vector.tensor_mul(t_es, ps[e0:e0 + 32, :], sin_t)
            nc.vector.tensor_mul(t_oc, ps[o0:o0 + 32, :], cos_t)
            nc.vector.tensor_add(dst_odd, t_es, t_oc)

        for qt in range(nqt):
            tsl = slice(qt * QTW, (qt + 1) * QTW)

            # ---- QKV projections for time tile qt ----
            xt = []
            for cb in range(8):
                xcb = xpool.tile([128, QTW], F32, tag="xt", name=f"xt{cb}_{qt}")
                nc.sync.dma_start(xcb, xT_r[:, cb, tsl])
                xt.append(xcb)
            cos_t = cpool.tile([HS // 2, QTW], F32, tag="cos", name=f"cos_{qt}")
            nc.sync.dma_start(cos_t, cos_d[:, tsl])
            sin_t = cpool.tile([HS // 2, QTW], F32, tag="sin", name=f"sin_{qt}")
            nc.sync.dma_start(sin_t, sin_d[:, tsl])

            qT_cur = []
            for pair in range(4):
                msl = slice(pair * 128, (pair + 1) * 128)
                psQ = psmm.tile([128, QTW], F32, tag="mm", name=f"psQ_{qt}_{pair}")
                for cb in range(8):
                    nc.tensor.matmul(psQ, lhsT=wq_sb[:, cb, msl], rhs=xt[cb],
                                     start=(cb == 0), stop=(cb == 7))
                qtile = qpool.tile([128, QTW], F32, tag="qT",
                                   name=f"qT_{qt}_{pair}")
                for g in range(2):
                    rope_write(qtile[64 * g:64 * g + 32, :],
                               qtile[64 * g + 32:64 * g + 64, :],
                               psQ, g, cos_t, sin_t)
                qT_cur.append(qtile)

                psK = psmm.tile([128, QTW], F32, tag="mm", name=f"psK_{qt}_{pair}")
                for cb in range(8):
                    nc.tensor.matmul(psK, lhsT=wk_sb[:, cb, msl], rhs=xt[cb],
                                     start=(cb == 0), stop=(cb == 7))
                for g in range(2):
                    rope_write(kT_sb[64 * g:64 * g + 32, pair, tsl],
                               kT_sb[64 * g + 32:64 * g + 64, pair, tsl],
                               psK, g, cos_t, sin_t)

            for j in range(4):
                tb = nkb_per_qt * qt + j
                psV = psmm.tile([128, QTW], F32, tag="mm", name=f"psV_{qt}_{j}")
                for cb in range(8):
                    nc.tensor.matmul(psV, lhsT=xt[cb][:, j * 128:(j + 1) * 128],
                                     rhs=wv_sb[:, cb, :],
                                     start=(cb == 0), stop=(cb == 7))
                nc.vector.tensor_copy(v_sb[:, tb, :, 0:HS],
                                      psV.rearrange("p (h d) -> p h d", h=HL))

            # ---- causal attention for query tile qt ----
            outT_cur = []
            for pair in range(4):
                nkb = (qt + 1) * nkb_per_qt
                psAB = []
                for half in range(2):
                    ps = psav.tile([HS + 1, QTW], F32, tag="av",
                                   name=f"psAV_{qt}_{pair}_{half}")
                    psAB.append(ps)
                for kb in range(nkb):
                    ksl = slice(kb * KBW, (kb + 1) * KBW)
                    for half in range(2):
                        r0 = 64 * half
                        psS = psmm.tile([KBW, QTW], F32, tag="mm",
                                        name=f"psS_{qt}_{pair}_{kb}_{half}")
                        nc.tensor.matmul(psS,
                                         lhsT=kT_sb[r0:r0 + 64, pair, ksl],
                                         rhs=qT_cur[pair][r0:r0 + 64, :],
                                         start=True, stop=True)
                        pT = ppool.tile([KBW, QTW], F32, tag="pT",
                                        name=f"pT_{qt}_{pair}_{kb}_{half}")
                        nc.scalar.activation(pT, psS, EXPF, scale=0.125)
                        if kb >= nkb - nkb_per_qt:
                            nc.gpsimd.affine_select(
                                pT, pT, pattern=[[1, QTW]],
                                compare_op=mybir.AluOpType.is_ge, fill=0.0,
                                base=QTW * qt - KBW * kb, channel_multiplier=-1)
                        h = 2 * pair + half
                        nc.tensor.matmul(psAB[half], lhsT=v_sb[:, kb, h, :],
                                         rhs=pT, start=(kb == 0),
                                         stop=(kb == nkb - 1))

                otile = opool.tile([128, QTW], F32, tag="outT",
                                   name=f"outT_{qt}_{pair}")
                for half in range(2):
                    rec = rpool.tile([1, QTW], F32, tag="rec",
                                     name=f"rec_{qt}_{pair}_{half}")
                    nc.vector.reciprocal(rec, psAB[half][HS:HS + 1, :])
                    psBC = psmm.tile([HS, QTW], F32, tag="mm",
                                     name=f"psBC_{qt}_{pair}_{half}")
                    nc.tensor.matmul(psBC, lhsT=ones_sb, rhs=rec,
                                     start=True, stop=True)
                    bc = bpool.tile([HS, QTW], F32, tag="bc",
                                    name=f"bc_{qt}_{pair}_{half}")
                    nc.scalar.copy(bc, psBC)
                    nc.vector.tensor_mul(otile[64 * half:64 * half + 64, :],
                                         psAB[half][0:HS, :], bc)
                outT_cur.append(otile)

            # ---- partial output projection for tile qt ----
            for m in range(8):
                psY = psmm.tile([128, QTW], F32, tag="mm", name=f"psY_{qt}_{m}")
                for pb in range(4):
                    nc.tensor.matmul(psY, lhsT=wp_sb[:, pb, m * 128:(m + 1) * 128],
                                     rhs=outT_cur[pb],
                                     start=(pb == 0), stop=(pb == 3))
                ysb = ypool.tile([128, QTW], F32, tag="ysb", name=f"ysb_{qt}_{m}")
                nc.vector.tensor_copy(ysb, psY)
                nc.sync.dma_start(yT_r[:, m, tsl], ysb)

    nc.compile()
    return nc


def rope_tables(t_len):
    inv = 1.0 / (THETA ** (np.arange(0, HS, 2, dtype=np.float64) / HS))
    ang = np.arange(t_len, dtype=np.float64)[:, None] * inv[None, :]
    cosT = np.ascontiguousarray(np.cos(ang).T.astype(np.float32))
    sinT = np.ascontiguousarray(np.sin(ang).T.astype(np.float32))
    return cosT, sinT


def head_perm():
    p = []
    for h in range(N_HEAD):
        base = h * HS
        p += list(range(base, base + HS, 2))
        p += list(range(base + 1, base + HS, 2))
    return np.asarray(p)


def make_in_maps(x, Wq, Wk, Wv, Wp, t_len=T):
    cosT, sinT = rope_tables(t_len)
    perm = head_perm()
    Wqp = np.ascontiguousarray(Wq[:, perm])
    Wkp = np.ascontiguousarray(Wk[:, perm])
    xTs = [np.ascontiguousarray(x[b].T) for b in range(x.shape[0])]
    in_maps = []
    for c in range(N_CORES):
        b, hg = divmod(c, 2)
        sl = slice(hg * DL, (hg + 1) * DL)
        in_maps.append({
            "xT": xTs[b],
            "wq": np.ascontiguousarray(Wqp[:, sl]),
            "wk": np.ascontiguousarray(Wkp[:, sl]),
            "wv": np.ascontiguousarray(Wv[:, sl]),
            "wp": np.ascontiguousarray(Wp[sl, :]),
            "cosT": cosT,
            "sinT": sinT,
        })
    return in_maps


def combine_outputs(per_core_yT, bp, t_len=T):
    y = np.empty((B, t_len, C), np.float32)
    for b in range(B):
        acc = per_core_yT[2 * b] + per_core_yT[2 * b + 1]
        y[b] = acc.T + bp
    return y


_NC_CACHE = {}


def kernel(x, Wq, Wk, Wv, Wp, bp):
    x = np.asarray(x, dtype=np.float32)
    Wq = np.asarray(Wq, dtype=np.float32)
    Wk = np.asarray(Wk, dtype=np.float32)
    Wv = np.asarray(Wv, dtype=np.float32)
    Wp = np.asarray(Wp, dtype=np.float32)
    bp = np.asarray(bp, dtype=np.float32)

    if "nc" not in _NC_CACHE:
        _NC_CACHE["nc"] = build_nc()
    nc = _NC_CACHE["nc"]

    in_maps = make_in_maps(x, Wq, Wk, Wv, Wp)
    res = run_bass_kernel_spmd(nc, in_maps, core_ids=list(range(N_CORES)))
    return combine_outputs([res.results[c]["yT"] for c in range(N_CORES)], bp)


# revision 9
# speedup vs baseline: 3.0922x; 3.0922x over previous
"""Multi-head causal attention with RoPE on 8 Trainium2 NeuronCores (Bass/Tile).

Problem: B=4, T=2048, C=1024, 16 heads x 64. y = softmax(rope(q) rope(k)^T / 8,
causal) v @ Wp + bp with q/k/v = x @ Wq/Wk/Wv.

Sharding: core c handles batch b=c//2, head-group hg=c%2 (8 heads). Each core
computes its QKV projections with column-sliced weights, flash-style causal
attention for its heads, and a partial output projection y^T = Wp_slice^T @
out^T. The host sums the two partials per batch (tensor-parallel reduce) and
adds the bias.

Device scheme (all layouts transposed so no on-device transposes are needed):
  xT (C,T), q^T/k^T (512,T), v (T, 8 heads x [64 dims | 1.0]), y^T (C,T f32).
  Matmul operands are float16 (1 PE cycle/row vs 4 for fp32 LOW_HIGH); all
  accumulation stays fp32 in PSUM, softmax/normalization math stays fp32.
  Wq/Wk columns are permuted per head to [even dims | odd dims] so RoPE becomes
  full-width elementwise ops: qT = raw*C1 + swap(raw)*C2, where swap exchanges
  the 32-partition x0/x1 groups (done with 4 small SBUF-to-SBUF DMAs) and
  C1/C2 are host-precomputed (128,T) cos/sin tables. Scores are computed as
  S^T (keys on partitions) with 2 heads row-packed per matmul (K=64 each); exp
  runs on ACT straight out of PSUM with the 1/8 scale folded in; causal
  masking is a post-exp affine_select (GPSIMD) on the 4 diagonal blocks; a
  ones column appended to v makes the AV matmul emit the softmax denominator
  as row 64 of the accumulator; reciprocals are batched (8,512) per query
  tile and broadcast over partitions with a K=1 matmul against a ones row.
"""
from contextlib import ExitStack

import numpy as np

import concourse.bacc as bacc
import concourse.bass as bass
import concourse.mybir as mybir
import concourse.tile as tile
from concourse.bass_utils import run_bass_kernel_spmd

N_HEAD = 16
HS = 64
C = 1024
B = 4
T = 2048
THETA = 10000.0
N_CORES = 8
HL = N_HEAD // 2          # heads per core
DL = HL * HS              # 512 local head dims
QTW = 512                 # query tile width
KBW = 128                 # key block width
F32 = mybir.dt.float32
F16 = mybir.dt.float16
EXPF = mybir.ActivationFunctionType.Exp


def build_nc(t_len=T):
    assert t_len % QTW == 0
    nqt = t_len // QTW
    nkb_per_qt = QTW // KBW  # 4

    nc = bacc.Bacc("TRN2", target_bir_lowering=False, debug=False,
                   num_devices=N_CORES)
    xT_d = nc.dram_tensor("xT", [C, t_len], F16, kind="ExternalInput")
    wq_d = nc.dram_tensor("wq", [C, DL], F16, kind="ExternalInput")
    wk_d = nc.dram_tensor("wk", [C, DL], F16, kind="ExternalInput")
    wv_d = nc.dram_tensor("wv", [C, DL], F16, kind="ExternalInput")
    wp_d = nc.dram_tensor("wp", [DL, C], F16, kind="ExternalInput")
    c1_d = nc.dram_tensor("c1", [128, t_len], F32, kind="ExternalInput")
    c2_d = nc.dram_tensor("c2", [128, t_len], F32, kind="ExternalInput")
    sel_d = nc.dram_tensor("sel", [HL, HL * HS], F16, kind="ExternalInput")
    yT_d = nc.dram_tensor("yT", [C, t_len], F32, kind="ExternalOutput")

    xT_r = xT_d.ap().rearrange("(cb p) t -> p cb t", p=128)    # (128, 8, t)
    wq_r = wq_d.ap().rearrange("(cb p) m -> p cb m", p=128)    # (128, 8, 512)
    wk_r = wk_d.ap().rearrange("(cb p) m -> p cb m", p=128)
    wv_r = wv_d.ap().rearrange("(cb p) m -> p cb m", p=128)
    wp_r = wp_d.ap().rearrange("(pb p) m -> p pb m", p=128)    # (128, 4, 1024)
    yT_r = yT_d.ap().rearrange("(mb p) t -> p mb t", p=128)    # (128, 8, t)

    with tile.TileContext(nc) as tc, ExitStack() as ctx:
        persist = ctx.enter_context(tc.tile_pool(name="persist", bufs=1))
        xpool = ctx.enter_context(tc.tile_pool(name="xpool", bufs=10))
        qpool = ctx.enter_context(tc.tile_pool(name="qpool", bufs=6))
        opool = ctx.enter_context(tc.tile_pool(name="opool", bufs=6))
        ppool = ctx.enter_context(tc.tile_pool(name="ppool", bufs=6))
        rpool = ctx.enter_context(tc.tile_pool(name="rpool", bufs=3))
        tpool = ctx.enter_context(tc.tile_pool(name="tpool", bufs=3))
        ypool = ctx.enter_context(tc.tile_pool(name="ypool", bufs=4))
        psmm = ctx.enter_context(tc.tile_pool(name="psmm", bufs=4, space="PSUM"))
        psav = ctx.enter_context(tc.tile_pool(name="psav", bufs=3, space="PSUM"))

        wq_sb = persist.tile([128, 8, DL], F16)
        nc.sync.dma_start(wq_sb, wq_r)
        wk_sb = persist.tile([128, 8, DL], F16)
        nc.sync.dma_start(wk_sb, wk_r)
        wv_sb = persist.tile([128, 8, DL], F16)
        nc.sync.dma_start(wv_sb, wv_r)
        wp_sb = persist.tile([128, 4, C], F16)
        nc.sync.dma_start(wp_sb, wp_r)
        c1_sb = persist.tile([128, t_len], F32)
        nc.sync.dma_start(c1_sb, c1_d.ap())
        c2_sb = persist.tile([128, t_len], F32)
        nc.sync.dma_start(c2_sb, c2_d.ap())

        kT_sb = persist.tile([128, 4, t_len], F16)           # rope'd k^T
        v_sb = persist.tile([128, t_len // KBW, HL, HS + 1], F16)
        nc.vector.memset(v_sb[:, :, :, HS], 1.0)             # denominator ones
        # one-hot selector: sel_sb[k, 64h+m] = (k == h); lhsT slice for pair p
        # broadcasts rec16 rows (2p, 2p+1) onto partitions [0:64)/[64:128).
        sel_sb = persist.tile([HL, HL * HS], F16)
        nc.sync.dma_start(sel_sb, sel_d.ap())

        def rope_write(dst, ps, tsl):
            """dst: (128, QTW) fp16 SBUF AP; ps: (128, QTW) f32 PSUM tile.
            dst = ps*C1 + swap32(ps)*C2 with swap32 exchanging the x0/x1
            32-partition groups inside each 64-partition head."""
            raw = tpool.tile([128, QTW], F32, tag="raw", name=f"rr_{nc.next_id()}")
            nc.vector.tensor_copy(raw, ps)
            swp = tpool.tile([128, QTW], F32, tag="swp", name=f"rs_{nc.next_id()}")
            for a, b in ((0, 32), (32, 0), (64, 96), (96, 64)):
                nc.sync.dma_start(swp[a:a + 32, :], raw[b:b + 32, :])
            t1 = tpool.tile([128, QTW], F32, tag="t1", name=f"r1_{nc.next_id()}")
            nc.vector.tensor_mul(t1, raw, c1_sb[:, tsl])
            t2 = tpool.tile([128, QTW], F32, tag="t2", name=f"r2_{nc.next_id()}")
            nc.vector.tensor_mul(t2, swp, c2_sb[:, tsl])
            nc.vector.tensor_add(dst, t1, t2)

        for qt in range(nqt):
            tsl = slice(qt * QTW, (qt + 1) * QTW)

            # ---- QKV projections for time tile qt ----
            xt = []
            for cb in range(8):
                xcb = xpool.tile([128, QTW], F16, tag="xt", name=f"xt{cb}_{qt}")
                nc.sync.dma_start(xcb, xT_r[:, cb, tsl])
                xt.append(xcb)

            qT_cur = []
            for pair in range(4):
                msl = slice(pair * 128, (pair + 1) * 128)
                psQ = psmm.tile([128, QTW], F32, tag="mm", name=f"psQ_{qt}_{pair}")
                for cb in range(8):
                    nc.tensor.matmul(psQ, lhsT=wq_sb[:, cb, msl], rhs=xt[cb],
                                     start=(cb == 0), stop=(cb == 7))
                qtile = qpool.tile([128, QTW], F16, tag="qT",
                                   name=f"qT_{qt}_{pair}")
                rope_write(qtile, psQ, tsl)
                qT_cur.append(qtile)

                psK = psmm.tile([128, QTW], F32, tag="mm", name=f"psK_{qt}_{pair}")
                for cb in range(8):
                    nc.tensor.matmul(psK, lhsT=wk_sb[:, cb, msl], rhs=xt[cb],
                                     start=(cb == 0), stop=(cb == 7))
                rope_write(kT_sb[:, pair, tsl], psK, tsl)

            for j in range(4):
                tb = nkb_per_qt * qt + j
                psV = psmm.tile([128, QTW], F32, tag="mm", name=f"psV_{qt}_{j}")
                for cb in range(8):
                    nc.tensor.matmul(psV, lhsT=xt[cb][:, j * 128:(j + 1) * 128],
                                     rhs=wv_sb[:, cb, :],
                                     start=(cb == 0), stop=(cb == 7))
                nc.vector.tensor_copy(v_sb[:, tb, :, 0:HS],
                                      psV.rearrange("p (h d) -> p h d", h=HL))

            # ---- causal attention for query tile qt ----
            den_all = rpool.tile([HL, QTW], F32, tag="den", name=f"den_{qt}")
            outT_cur = []
            for pair in range(4):
                nkb = (qt + 1) * nkb_per_qt
                psAB = []
                for half in range(2):
                    ps = psav.tile([HS + 1, QTW], F32, tag="av",
                                   name=f"psAV_{qt}_{pair}_{half}")
                    psAB.append(ps)
                for kb in range(nkb):
                    ksl = slice(kb * KBW, (kb + 1) * KBW)
                    for half in range(2):
                        r0 = 64 * half
                        psS = psmm.tile([KBW, QTW], F32, tag="mm",
                                        name=f"psS_{qt}_{pair}_{kb}_{half}")
                        nc.tensor.matmul(psS,
                                         lhsT=kT_sb[r0:r0 + 64, pair, ksl],
                                         rhs=qT_cur[pair][r0:r0 + 64, :],
                                         start=True, stop=True)
                        pT = ppool.tile([KBW, QTW], F16, tag="pT",
                                        name=f"pT_{qt}_{pair}_{kb}_{half}")
                        nc.scalar.activation(pT, psS, EXPF, scale=0.125)
                        if kb >= nkb - nkb_per_qt:
                            nc.gpsimd.affine_select(
                                pT, pT, pattern=[[1, QTW]],
                                compare_op=mybir.AluOpType.is_ge, fill=0.0,
                                base=QTW * qt - KBW * kb, channel_multiplier=-1)
                        h = 2 * pair + half
                        nc.tensor.matmul(psAB[half], lhsT=v_sb[:, kb, h, :],
                                         rhs=pT, start=(kb == 0),
                                         stop=(kb == nkb - 1))

                otile = opool.tile([128, QTW], F16, tag="outT",
                                   name=f"outT_{qt}_{pair}")
                for half in range(2):
                    h = 2 * pair + half
                    # engines need 32-aligned base partitions; bounce the
                    # denominator row through partition 0, DMA into row h
                    dtmp = rpool.tile([1, QTW], F32, tag="dtmp",
                                      name=f"dtmp_{qt}_{h}")
                    nc.vector.tensor_copy(dtmp, psAB[half][HS:HS + 1, :])
                    nc.sync.dma_start(den_all[h:h + 1, :], dtmp)
                    nc.vector.tensor_copy(otile[64 * half:64 * half + 64, :],
                                          psAB[half][0:HS, :])
                outT_cur.append(otile)

            rec = rpool.tile([HL, QTW], F32, tag="rec", name=f"rec_{qt}")
            nc.vector.reciprocal(rec, den_all)
            rec16 = rpool.tile([HL, QTW], F16, tag="rec16", name=f"rec16_{qt}")
            nc.vector.tensor_copy(rec16, rec)
            for pair in range(4):
                psBC = psmm.tile([128, QTW], F32, tag="mm",
                                 name=f"psBC_{qt}_{pair}")
                nc.tensor.matmul(psBC, lhsT=sel_sb[:, 128 * pair:128 * (pair + 1)],
                                 rhs=rec16, start=True, stop=True)
                nc.vector.tensor_mul(outT_cur[pair], outT_cur[pair], psBC)

            # ---- partial output projection for tile qt ----
            for m in range(8):
                psY = psmm.tile([128, QTW], F32, tag="mm", name=f"psY_{qt}_{m}")
                for pb in range(4):
                    nc.tensor.matmul(psY, lhsT=wp_sb[:, pb, m * 128:(m + 1) * 128],
                                     rhs=outT_cur[pb],
                                     start=(pb == 0), stop=(pb == 3))
                ysb = ypool.tile([128, QTW], F32, tag="ysb", name=f"ysb_{qt}_{m}")
                nc.vector.tensor_copy(ysb, psY)
                nc.sync.dma_start(yT_r[:, m, tsl], ysb)

    nc.compile()
    return nc


def rope_tables(t_len):
    """C1/C2 (128, t_len) fp32: per 64-partition head-half, rows [0:32]=x0
    slots, [32:64]=x1 slots. roped = raw*C1 + swap32(raw)*C2."""
    inv = 1.0 / (THETA ** (np.arange(0, HS, 2, dtype=np.float64) / HS))
    ang = np.arange(t_len, dtype=np.float64)[:, None] * inv[None, :]
    cosT = np.cos(ang).T.astype(np.float32)   # (32, t)
    sinT = np.sin(ang).T.astype(np.float32)
    c1 = np.tile(cosT, (4, 1))                              # cos everywhere
    c2 = np.concatenate([-sinT, sinT, -sinT, sinT], axis=0)
    return np.ascontiguousarray(c1), np.ascontiguousarray(c2)


def head_perm():
    p = []
    for h in range(N_HEAD):
        base = h * HS
        p += list(range(base, base + HS, 2))
        p += list(range(base + 1, base + HS, 2))
    return np.asarray(p)


def make_in_maps(x, Wq, Wk, Wv, Wp, t_len=T):
    c1, c2 = rope_tables(t_len)
    sel = np.zeros((HL, HL * HS), np.float16)
    for h in range(HL):
        sel[h, HS * h:HS * (h + 1)] = 1.0
    perm = head_perm()
    Wqp = np.ascontiguousarray(Wq[:, perm].astype(np.float16))
    Wkp = np.ascontiguousarray(Wk[:, perm].astype(np.float16))
    Wv16 = Wv.astype(np.float16)
    Wp16 = Wp.astype(np.float16)
    xTs = [np.ascontiguousarray(x[b].T.astype(np.float16))
           for b in range(x.shape[0])]
    in_maps = []
    for cidx in range(N_CORES):
        b, hg = divmod(cidx, 2)
        sl = slice(hg * DL, (hg + 1) * DL)
        in_maps.append({
            "xT": xTs[b],
            "wq": np.ascontiguousarray(Wqp[:, sl]),
            "wk": np.ascontiguousarray(Wkp[:, sl]),
            "wv": np.ascontiguousarray(Wv16[:, sl]),
            "wp": np.ascontiguousarray(Wp16[sl, :]),
            "c1": c1,
            "c2": c2,
            "sel": sel,
        })
    return in_maps


def combine_outputs(per_core_yT, bp, t_len=T):
    y = np.empty((B, t_len, C), np.float32)
    for b in range(B):
        acc = per_core_yT[2 * b] + per_core_yT[2 * b + 1]
        y[b] = acc.T + bp
    return y


_NC_CACHE = {}


def kernel(x, Wq, Wk, Wv, Wp, bp):
    x = np.asarray(x, dtype=np.float32)
    Wq = np.asarray(Wq, dtype=np.float32)
    Wk = np.asarray(Wk, dtype=np.float32)
    Wv = np.asarray(Wv, dtype=np.float32)
    Wp = np.asarray(Wp, dtype=np.float32)
    bp = np.asarray(bp, dtype=np.float32)

    if "nc" not in _NC_CACHE:
        _NC_CACHE["nc"] = build_nc()
    nc = _NC_CACHE["nc"]

    in_maps = make_in_maps(x, Wq, Wk, Wv, Wp)
    res = run_bass_kernel_spmd(nc, in_maps, core_ids=list(range(N_CORES)))
    return combine_outputs([res.results[c]["yT"] for c in range(N_CORES)], bp)


# revision 11
# speedup vs baseline: 3.3766x; 1.0920x over previous
"""Multi-head causal attention with RoPE on 8 Trainium2 NeuronCores (Bass/Tile).

Problem: B=4, T=2048, C=1024, 16 heads x 64. y = softmax(rope(q) rope(k)^T / 8,
causal) v @ Wp + bp with q/k/v = x @ Wq/Wk/Wv.

Sharding: core c handles batch b=c//2, head-group hg=c%2 (8 heads). Each core
computes its QKV projections with column-sliced weights, flash-style causal
attention for its heads, and a partial output projection y^T = Wp_slice^T @
out^T. The host sums the two partials per batch (tensor-parallel reduce) and
adds the bias.

Device scheme (all layouts transposed so no on-device transposes are needed):
  xT (C,T), q^T/k^T (512,T), v (T, 8 heads x [64 dims | 1.0]), y^T (C,T f32).
  Matmul operands are float16 (1 PE cycle/row vs 4 for fp32 LOW_HIGH); all
  accumulation stays fp32 in PSUM, softmax/normalization math stays fp32.
  Wq/Wk columns are permuted per head to [even dims | odd dims] so RoPE becomes
  full-width elementwise ops: qT = raw*C1 + swap(raw)*C2, where swap exchanges
  the 32-partition x0/x1 groups (done with 4 small SBUF-to-SBUF DMAs) and
  C1/C2 are host-precomputed (128,T) cos/sin tables. Scores are computed as
  S^T (keys on partitions) with 2 heads row-packed per matmul (K=64 each); exp
  runs on ACT straight out of PSUM with the 1/8 scale folded in; causal
  masking is a post-exp affine_select (GPSIMD) on the 4 diagonal blocks; a
  ones column appended to v makes the AV matmul emit the softmax denominator
  as row 64 of the accumulator; reciprocals are batched (8,512) per query
  tile and broadcast over partitions with a K=1 matmul against a ones row.
"""
from contextlib import ExitStack

import numpy as np

import concourse.bacc as bacc
import concourse.bass as bass
import concourse.mybir as mybir
import concourse.tile as tile
from concourse.bass_utils import run_bass_kernel_spmd

N_HEAD = 16
HS = 64
C = 1024
B = 4
T = 2048
THETA = 10000.0
N_CORES = 8
HL = N_HEAD // 2          # heads per core
DL = HL * HS              # 512 local head dims
QTW = 512                 # query tile width
KBW = 128                 # key block width
F32 = mybir.dt.float32
F16 = mybir.dt.float16
EXPF = mybir.ActivationFunctionType.Exp


def build_nc(t_len=T):
    assert t_len % QTW == 0
    nqt = t_len // QTW
    nkb_per_qt = QTW // KBW  # 4

    nc = bacc.Bacc("TRN2", target_bir_lowering=False, debug=False,
                   num_devices=N_CORES)
    xT_d = nc.dram_tensor("xT", [C, t_len], F16, kind="ExternalInput")
    wq_d = nc.dram_tensor("wq", [C, DL], F16, kind="ExternalInput")
    wk_d = nc.dram_tensor("wk", [C, DL], F16, kind="ExternalInput")
    wv_d = nc.dram_tensor("wv", [C, DL], F16, kind="ExternalInput")
    wp_d = nc.dram_tensor("wp", [DL, C], F16, kind="ExternalInput")
    c1_d = nc.dram_tensor("c1", [128, t_len], F32, kind="ExternalInput")
    c2_d = nc.dram_tensor("c2", [128, t_len], F32, kind="ExternalInput")
    sel_d = nc.dram_tensor("sel", [HL, HL * HS], F16, kind="ExternalInput")
    yT_d = nc.dram_tensor("yT", [C, t_len], F32, kind="ExternalOutput")

    xT_r = xT_d.ap().rearrange("(cb p) t -> p cb t", p=128)    # (128, 8, t)
    wq_r = wq_d.ap().rearrange("(cb p) m -> p cb m", p=128)    # (128, 8, 512)
    wk_r = wk_d.ap().rearrange("(cb p) m -> p cb m", p=128)
    wv_r = wv_d.ap().rearrange("(cb p) m -> p cb m", p=128)
    wp_r = wp_d.ap().rearrange("(pb p) m -> p pb m", p=128)    # (128, 4, 1024)
    yT_r = yT_d.ap().rearrange("(mb p) t -> p mb t", p=128)    # (128, 8, t)

    with tile.TileContext(nc) as tc, ExitStack() as ctx:
        persist = ctx.enter_context(tc.tile_pool(name="persist", bufs=1))
        xpool = ctx.enter_context(tc.tile_pool(name="xpool", bufs=2))
        qpool = ctx.enter_context(tc.tile_pool(name="qpool", bufs=6))
        opool = ctx.enter_context(tc.tile_pool(name="opool", bufs=6))
        ppool = ctx.enter_context(tc.tile_pool(name="ppool", bufs=8))
        rpool = ctx.enter_context(tc.tile_pool(name="rpool", bufs=3))
        tpool = ctx.enter_context(tc.tile_pool(name="tpool", bufs=3))
        ypool = ctx.enter_context(tc.tile_pool(name="ypool", bufs=4))
        psmm = ctx.enter_context(tc.tile_pool(name="psmm", bufs=5, space="PSUM"))
        psav = ctx.enter_context(tc.tile_pool(name="psav", bufs=3, space="PSUM"))

        wq_sb = persist.tile([128, 8, DL], F16)
        nc.sync.dma_start(wq_sb, wq_r)
        wk_sb = persist.tile([128, 8, DL], F16)
        nc.sync.dma_start(wk_sb, wk_r)
        wv_sb = persist.tile([128, 8, DL], F16)
        nc.sync.dma_start(wv_sb, wv_r)
        wp_sb = persist.tile([128, 4, C], F16)
        nc.sync.dma_start(wp_sb, wp_r)
        c1_sb = persist.tile([128, t_len], F32)
        nc.sync.dma_start(c1_sb, c1_d.ap())
        c2_sb = persist.tile([128, t_len], F32)
        nc.sync.dma_start(c2_sb, c2_d.ap())

        kT_sb = persist.tile([128, 4, t_len], F16)           # rope'd k^T
        v_sb = persist.tile([128, t_len // KBW, HL, HS + 1], F16)
        nc.vector.memset(v_sb[:, :, :, HS], 1.0)             # denominator ones
        # one-hot selector: sel_sb[k, 64h+m] = (k == h); lhsT slice for pair p
        # broadcasts rec16 rows (2p, 2p+1) onto partitions [0:64)/[64:128).
        sel_sb = persist.tile([HL, HL * HS], F16)
        nc.sync.dma_start(sel_sb, sel_d.ap())

        def rope_write(dst, ps, tsl):
            """dst: (128, QTW) fp16 SBUF AP; ps: (128, QTW) f32 PSUM tile.
            dst = ps*C1 + swap32(ps)*C2 with swap32 exchanging the x0/x1
            32-partition groups inside each 64-partition head."""
            raw = tpool.tile([128, QTW], F32, tag="raw", name=f"rr_{nc.next_id()}")
            nc.vector.tensor_copy(raw, ps)
            swp = tpool.tile([128, QTW], F32, tag="swp", name=f"rs_{nc.next_id()}")
            for a, b in ((0, 32), (32, 0), (64, 96), (96, 64)):
                nc.sync.dma_start(swp[a:a + 32, :], raw[b:b + 32, :])
            t1 = tpool.tile([128, QTW], F32, tag="t1", name=f"r1_{nc.next_id()}")
            nc.vector.tensor_mul(t1, raw, c1_sb[:, tsl])
            t2 = tpool.tile([128, QTW], F32, tag="t2", name=f"r2_{nc.next_id()}")
            nc.vector.tensor_mul(t2, swp, c2_sb[:, tsl])
            nc.vector.tensor_add(dst, t1, t2)

        for qt in range(nqt):
            tsl = slice(qt * QTW, (qt + 1) * QTW)

            # ---- QKV projections for time tile qt ----
            xt3 = xpool.tile([128, 8, QTW], F16, tag="xt", name=f"xt_{qt}")
            nc.sync.dma_start(xt3, xT_r[:, :, tsl])
            xt = [xt3[:, cb, :] for cb in range(8)]

            qT_cur = []
            for pair in range(4):
                msl = slice(pair * 128, (pair + 1) * 128)
                psQ = psmm.tile([128, QTW], F32, tag="mm", name=f"psQ_{qt}_{pair}")
                for cb in range(8):
                    nc.tensor.matmul(psQ, lhsT=wq_sb[:, cb, msl], rhs=xt[cb],
                                     start=(cb == 0), stop=(cb == 7))
                qtile = qpool.tile([128, QTW], F16, tag="qT",
                                   name=f"qT_{qt}_{pair}")
                rope_write(qtile, psQ, tsl)
                qT_cur.append(qtile)

                psK = psmm.tile([128, QTW], F32, tag="mm", name=f"psK_{qt}_{pair}")
                for cb in range(8):
                    nc.tensor.matmul(psK, lhsT=wk_sb[:, cb, msl], rhs=xt[cb],
                                     start=(cb == 0), stop=(cb == 7))
                rope_write(kT_sb[:, pair, tsl], psK, tsl)

            for j in range(4):
                tb = nkb_per_qt * qt + j
                psV = psmm.tile([128, QTW], F32, tag="mm", name=f"psV_{qt}_{j}")
                for cb in range(8):
                    nc.tensor.matmul(psV, lhsT=xt[cb][:, j * 128:(j + 1) * 128],
                                     rhs=wv_sb[:, cb, :],
                                     start=(cb == 0), stop=(cb == 7))
                nc.vector.tensor_copy(v_sb[:, tb, :, 0:HS],
                                      psV.rearrange("p (h d) -> p h d", h=HL))

            # ---- causal attention for query tile qt ----
            den_all = rpool.tile([HL, QTW], F32, tag="den", name=f"den_{qt}")
            outT_cur = []
            for pair in range(4):
                nkb = (qt + 1) * nkb_per_qt
                psAB = []
                for half in range(2):
                    ps = psav.tile([HS + 1, QTW], F32, tag="av",
                                   name=f"psAV_{qt}_{pair}_{half}")
                    psAB.append(ps)
                def emit_scores(kb):
                    """Scores matmuls for both packed heads; diagonal blocks
                    only compute the valid q-suffix [128d:512)."""
                    d = kb - (nkb - nkb_per_qt)
                    off = KBW * d if d > 0 else 0
                    ksl = slice(kb * KBW, (kb + 1) * KBW)
                    out = []
                    for half in range(2):
                        r0 = 64 * half
                        psS = psmm.tile([KBW, QTW], F32, tag="mm",
                                        name=f"psS_{qt}_{pair}_{kb}_{half}")
                        nc.tensor.matmul(psS[:, off:],
                                         lhsT=kT_sb[r0:r0 + 64, pair, ksl],
                                         rhs=qT_cur[pair][r0:r0 + 64, off:],
                                         start=True, stop=True)
                        out.append(psS)
                    return kb, off, out

                def emit_pv(kb, off, psS_pair):
                    diag = kb >= nkb - nkb_per_qt
                    for half in range(2):
                        pT = ppool.tile([KBW, QTW], F16, tag="pT",
                                        name=f"pT_{qt}_{pair}_{kb}_{half}")
                        nc.scalar.activation(pT[:, off:], psS_pair[half][:, off:],
                                             EXPF, scale=0.125)
                        if diag:
                            # valid iff local column index >= partition index
                            nc.gpsimd.affine_select(
                                pT[:, off:], pT[:, off:],
                                pattern=[[1, QTW - off]],
                                compare_op=mybir.AluOpType.is_ge, fill=0.0,
                                base=0, channel_multiplier=-1)
                        h = 2 * pair + half
                        nc.tensor.matmul(psAB[half][:, off:],
                                         lhsT=v_sb[:, kb, h, :],
                                         rhs=pT[:, off:], start=(kb == 0),
                                         stop=(kb == nkb - 1))

                prev = None
                for kb in range(nkb):
                    cur = emit_scores(kb)
                    if prev is not None:
                        emit_pv(*prev)
                    prev = cur
                emit_pv(*prev)

                otile = opool.tile([128, QTW], F16, tag="outT",
                                   name=f"outT_{qt}_{pair}")
                for half in range(2):
                    h = 2 * pair + half
                    # engines need 32-aligned base partitions; bounce the
                    # denominator row through partition 0, DMA into row h
                    dtmp = rpool.tile([1, QTW], F32, tag="dtmp",
                                      name=f"dtmp_{qt}_{h}")
                    nc.vector.tensor_copy(dtmp, psAB[half][HS:HS + 1, :])
                    nc.sync.dma_start(den_all[h:h + 1, :], dtmp)
                    nc.vector.tensor_copy(otile[64 * half:64 * half + 64, :],
                                          psAB[half][0:HS, :])
                outT_cur.append(otile)

            rec = rpool.tile([HL, QTW], F32, tag="rec", name=f"rec_{qt}")
            nc.vector.reciprocal(rec, den_all)
            rec16 = rpool.tile([HL, QTW], F16, tag="rec16", name=f"rec16_{qt}")
            nc.vector.tensor_copy(rec16, rec)
            for pair in range(4):
                psBC = psmm.tile([128, QTW], F32, tag="mm",
                                 name=f"psBC_{qt}_{pair}")
                nc.tensor.matmul(psBC, lhsT=sel_sb[:, 128 * pair:128 * (pair + 1)],
                                 rhs=rec16, start=True, stop=True)
                nc.vector.tensor_mul(outT_cur[pair], outT_cur[pair], psBC)

            # ---- partial output projection for tile qt ----
            for m in range(8):
                psY = psmm.tile([128, QTW], F32, tag="mm", name=f"psY_{qt}_{m}")
                for pb in range(4):
                    nc.tensor.matmul(psY, lhsT=wp_sb[:, pb, m * 128:(m + 1) * 128],
                                     rhs=outT_cur[pb],
                                     start=(pb == 0), stop=(pb == 3))
                ysb = ypool.tile([128, QTW], F32, tag="ysb", name=f"ysb_{qt}_{m}")
                nc.vector.tensor_copy(ysb, psY)
                nc.sync.dma_start(yT_r[:, m, tsl], ysb)

    nc.compile()
    return nc


def rope_tables(t_len):
    """C1/C2 (128, t_len) fp32: per 64-partition head-half, rows [0:32]=x0
    slots, [32:64]=x1 slots. roped = raw*C1 + swap32(raw)*C2."""
    inv = 1.0 / (THETA ** (np.arange(0, HS, 2, dtype=np.float64) / HS))
    ang = np.arange(t_len, dtype=np.float64)[:, None] * inv[None, :]
    cosT = np.cos(ang).T.astype(np.float32)   # (32, t)
    sinT = np.sin(ang).T.astype(np.float32)
    c1 = np.tile(cosT, (4, 1))                              # cos everywhere
    c2 = np.concatenate([-sinT, sinT, -sinT, sinT], axis=0)
    return np.ascontiguousarray(c1), np.ascontiguousarray(c2)


def head_perm():
    p = []
    for h in range(N_HEAD):
        base = h * HS
        p += list(range(base, base + HS, 2))
        p += list(range(base + 1, base + HS, 2))
    return np.asarray(p)


def make_in_maps(x, Wq, Wk, Wv, Wp, t_len=T):
    c1, c2 = rope_tables(t_len)
    sel = np.zeros((HL, HL * HS), np.float16)
    for h in range(HL):
        sel[h, HS * h:HS * (h + 1)] = 1.0
    perm = head_perm()
    Wqp = np.ascontiguousarray(Wq[:, perm].astype(np.float16))
    Wkp = np.ascontiguousarray(Wk[:, perm].astype(np.float16))
    Wv16 = Wv.astype(np.float16)
    Wp16 = Wp.astype(np.float16)
    xTs = [np.ascontiguousarray(x[b].T.astype(np.float16))
           for b in range(x.shape[0])]
    in_maps = []
    for cidx in range(N_CORES):
        b, hg = divmod(cidx, 2)
        sl = slice(hg * DL, (hg + 1) * DL)
        in_maps.append({
            "xT": xTs[b],
            "wq": np.ascontiguousarray(Wqp[:, sl]),
            "wk": np.ascontiguousarray(Wkp[:, sl]),
            "wv": np.ascontiguousarray(Wv16[:, sl]),
            "wp": np.ascontiguousarray(Wp16[sl, :]),
            "c1": c1,
            "c2": c2,
            "sel": sel,
        })
    return in_maps


def combine_outputs(per_core_yT, bp, t_len=T):
    y = np.empty((B, t_len, C), np.float32)
    for b in range(B):
        acc = per_core_yT[2 * b] + per_core_yT[2 * b + 1]
        y[b] = acc.T + bp
    return y


_NC_CACHE = {}


def kernel(x, Wq, Wk, Wv, Wp, bp):
    x = np.asarray(x, dtype=np.float32)
    Wq = np.asarray(Wq, dtype=np.float32)
    Wk = np.asarray(Wk, dtype=np.float32)
    Wv = np.asarray(Wv, dtype=np.float32)
    Wp = np.asarray(Wp, dtype=np.float32)
    bp = np.asarray(bp, dtype=np.float32)

    if "nc" not in _NC_CACHE:
        _NC_CACHE["nc"] = build_nc()
    nc = _NC_CACHE["nc"]

    in_maps = make_in_maps(x, Wq, Wk, Wv, Wp)
    res = run_bass_kernel_spmd(nc, in_maps, core_ids=list(range(N_CORES)))
    return combine_outputs([res.results[c]["yT"] for c in range(N_CORES)], bp)
